# revision 6
# baseline (speedup 1.0000x reference)
"""Trainium2 Bass kernel for nn_Conv2d_24833500905755 (3x3 conv, B=32,
C_in=64, C_out=128, 56x56, pad 1, with the reference's mismatched
weight-flatten order).

Math: out[b,co,h,w] = sum_{c,di,dj} xpad[b,c,h+di,w+dj] * Wt[c,di*3+dj,co]
with Wt = K.reshape(576, C_OUT).reshape(C_IN, 9, C_OUT).

Data-parallel: 4 images per NeuronCore, 2 images packed on the
128-partition dim (fp16 matmuls, K=64 contraction per half, concurrent
PE row-group tiles). Raw-bass hand-scheduled engine programs.

v2 schedule (vs v1): input + weights split across BOTH hardware DMA
queues by partition half (sync=partitions 0:64, scalar=64:128) with
fine-grained gating (w split per k-group so the first chunk can start
before the whole weight tensor lands); outputs staged per-image in
SBUF and DMA'd in 16-row batches as soon as their chunks are copied
(keeps the queues continuously busy instead of bursty); no trailing
all-engine barrier (the NEFF postamble has its own rendezvous) - only
sync waits for output-DMA completion.

  Sync:   h0 input DMAs, h0 output batch DMAs, final s_out wait
  Scalar: h1 input DMAs, h1 PSUM->SBUF copies, h1 output batch DMAs
  Tensor: 4 junk warm-up pairs + 252 fp16 matmuls
  Vector: h0 PSUM->SBUF copies
"""

from contextlib import ExitStack

import numpy as np

import concourse.bass as bass
import concourse.mybir as mybir
from concourse.bass import BassBlock
from concourse.bass_utils import run_bass_kernel_spmd

B, C_IN, C_OUT, H = 32, 64, 128, 56
KS = 3
N_CORES = 8
BPC = B // N_CORES
HP = H + 2
RCHUNK = 8
NCHUNK = H // RCHUNK          # 7 chunks/image, 14 global chunk-pairs
MM_DT = mybir.dt.float16

# x row pieces per pair: piece i covers rows [XPIECES[i], XPIECES[i+1])
XPIECES = [0, 10, 36, HP]
# chunk ci needs input rows <= ci*8+10; piece gate index per chunk
CHUNK_PIECE = [0, 1, 1, 1, 2, 2, 2]
# w split by kernel position: [0:3) then [3:9)
WSPLIT = 3
# output batches (row ranges) per image; last batch kept small for the tail
OBATCH = [(0, 16), (16, 32), (32, 48), (48, 56)]
N_OUT_DMAS = BPC * len(OBATCH)  # per-image batches, one DMA each


class NoBarrierBlock(BassBlock):
    """BassBlock without the exit-time all-engine barrier/drain: the
    compiler-emitted postamble performs its own all-engine rendezvous
    before touching semaphores, so the extra barrier only adds latency.
    Engine streams simply branch to the common end block."""

    def __exit__(self, exc_type, exc_val, exc_tb):
        if exc_type is None:
            for engine, last_body in self.last_body.items():
                with self.bass.body(
                    last_body, parent=self.bass.cur_bb, allow_existing_parent=True
                ):
                    engine.br(self.end_bb)
            self.bass.switch_bb(self.end_bb)


def build_nc(mm_dt=MM_DT):
    f32 = mybir.dt.float32
    nc = bass.Bass()
    x_ext = nc.declare_dram_parameter("x", [BPC, C_IN, HP, HP], mm_dt, isOutput=False)
    w_ext = nc.declare_dram_parameter("w", [2 * C_IN, KS * KS, C_OUT], mm_dt, isOutput=False)
    out_ext = nc.declare_dram_parameter("out", [BPC, C_OUT, H, H], f32, isOutput=True)

    with ExitStack() as ctx:
        wt = ctx.enter_context(nc.sbuf_tensor("wt", [2 * C_IN, KS * KS, C_OUT], mm_dt))
        xps = [
            ctx.enter_context(nc.sbuf_tensor(f"xp{p}", [2 * C_IN, HP, HP], mm_dt))
            for p in range(2)
        ]
        # per-image output staging: ob[img] = [C_OUT, H, H] f32
        obs = [
            ctx.enter_context(nc.sbuf_tensor(f"ob{b}", [C_OUT, H, H], f32))
            for b in range(BPC)
        ]
        # banks[slot][half] - 8 PSUM banks
        banks = [
            [
                ctx.enter_context(
                    nc.psum_tensor(f"ps_{s}_{h}", [C_OUT, RCHUNK, H], f32)
                )
                for h in range(2)
            ]
            for s in range(4)
        ]
        s_w1 = ctx.enter_context(nc.semaphore("s_w1"))
        s_w2 = ctx.enter_context(nc.semaphore("s_w2"))
        # one sem per pair-0 row piece: both queue-halves inc by 16, so a
        # piece is fully resident at >= 32 (a shared counter would race -
        # one queue running two pieces ahead could fake the other's arrival)
        s_x0p = [
            ctx.enter_context(nc.semaphore(f"s_x0p{i}"))
            for i in range(len(XPIECES) - 1)
        ]
        s_x1 = ctx.enter_context(nc.semaphore("s_x1"))
        s_mm = ctx.enter_context(nc.semaphore("s_mm"))
        s_cpv = ctx.enter_context(nc.semaphore("s_cpv"))   # h0 copies (vector)
        s_cph = ctx.enter_context(nc.semaphore("s_cph"))   # h1 copies (scalar)
        s_out = ctx.enter_context(nc.semaphore("s_out"))

        def in_dmas(eng, h):
            """Input DMAs for partition half h on engine eng's queue.
            Queue order: w[0:3) -> x0 piece0 -> w[3:9) -> x0 pieces 1,2 ->
            x1, so chunk 0 can start on (w1, piece0) while the rest lands."""
            c0 = h * C_IN
            src0 = x_ext[h : h + 1].rearrange("b c h w -> (b c) h w")
            src1 = x_ext[2 + h : 3 + h].rearrange("b c h w -> (b c) h w")
            eng.dma_start(
                out=wt[c0 : c0 + C_IN, 0:WSPLIT, :], in_=w_ext[c0 : c0 + C_IN, 0:WSPLIT, :]
            ).then_inc(s_w1, 16)
            lo, hi = XPIECES[0], XPIECES[1]
            eng.dma_start(
                out=xps[0][c0 : c0 + C_IN, lo:hi, :], in_=src0[:, lo:hi, :]
            ).then_inc(s_x0p[0], 16)
            eng.dma_start(
                out=wt[c0 : c0 + C_IN, WSPLIT : KS * KS, :],
                in_=w_ext[c0 : c0 + C_IN, WSPLIT : KS * KS, :],
            ).then_inc(s_w2, 16)
            for i in range(1, len(XPIECES) - 1):
                lo, hi = XPIECES[i], XPIECES[i + 1]
                eng.dma_start(
                    out=xps[0][c0 : c0 + C_IN, lo:hi, :], in_=src0[:, lo:hi, :]
                ).then_inc(s_x0p[i], 16)
            eng.dma_start(
                out=xps[1][c0 : c0 + C_IN, :, :], in_=src1[:, :, :]
            ).then_inc(s_x1, 16)

        def out_dmas(eng, h, cp_sem):
            """Output batch DMAs for images of half h, gated on copies."""
            for p in range(2):
                img = 2 * p + h
                dst = out_ext[img : img + 1].rearrange("b c h w -> (b c) h w")
                for (blo, bhi) in OBATCH:
                    last_chunk = p * NCHUNK + (bhi - 1) // RCHUNK
                    eng.wait_ge(cp_sem, last_chunk + 1)
                    eng.dma_start(
                        out=dst[:, blo:bhi, :], in_=obs[img][:, blo:bhi, :]
                    ).then_inc(s_out, 16)

        with NoBarrierBlock(nc, "blk") as block:

            @block.sync
            def _(sync: bass.BassEngine):
                in_dmas(sync, 0)
                out_dmas(sync, 0, s_cpv)
                sync.wait_ge(s_out, 16 * N_OUT_DMAS)

            @block.scalar
            def _(scalar: bass.BassEngine):
                in_dmas(scalar, 1)
                # h1 copies interleaved with h1 output issues (program order
                # on this engine keeps copy -> dma correctly ordered)
                for p in range(2):
                    img = 2 * p + 1
                    dst = out_ext[img : img + 1].rearrange("b c h w -> (b c) h w")
                    bi = 0
                    for ci in range(NCHUNK):
                        c = p * NCHUNK + ci
                        h0 = ci * RCHUNK
                        scalar.wait_ge(s_mm, c + 1)
                        scalar.copy(
                            out=obs[img][:, h0 : h0 + RCHUNK, :],
                            in_=banks[c % 4][1][:],
                        ).then_inc(s_cph, 1)
                        blo, bhi = OBATCH[bi]
                        if h0 + RCHUNK == bhi:
                            scalar.dma_start(
                                out=dst[:, blo:bhi, :], in_=obs[img][:, blo:bhi, :]
                            ).then_inc(s_out, 16)
                            bi += 1

            @block.tensor
            def _(tensor: bass.BassEngine):
                # A few junk warm-up pairs while the first input piece lands.
                for wi in range(8):
                    h = wi % 2
                    c0 = h * C_IN
                    tensor.matmul(
                        out=banks[3][h][:],
                        lhsT=wt[c0 : c0 + C_IN, 0, :],
                        rhs=xps[0][c0 : c0 + C_IN, 0:RCHUNK, 0:H],
                        start=True,
                        stop=True,
                    )
                for p in range(2):
                    for ci in range(NCHUNK):
                        c = p * NCHUNK + ci
                        h0 = ci * RCHUNK
                        if p == 0:
                            if ci == 0:
                                tensor.wait_ge(s_w1, 32)
                                tensor.wait_ge(s_x0p[0], 32)
                            elif CHUNK_PIECE[ci] > CHUNK_PIECE[ci - 1]:
                                tensor.wait_ge(s_x0p[CHUNK_PIECE[ci]], 32)
                        else:
                            if ci == 0:
                                tensor.wait_ge(s_x1, 32)
                        if c >= 4:
                            # WAR: bank slot c%4 last used by chunk c-4
                            tensor.wait_ge(s_cpv, c - 3)
                            tensor.wait_ge(s_cph, c - 3)
                        for k in range(KS * KS):
                            di, dj = divmod(k, KS)
                            if c == 0 and k == WSPLIT:
                                tensor.wait_ge(s_w2, 32)
                            last = k == KS * KS - 1
                            for half in range(2):
                                c0 = half * C_IN
                                mm = tensor.matmul(
                                    out=banks[c % 4][half][:],
                                    lhsT=wt[c0 : c0 + C_IN, k, :],
                                    rhs=xps[p][
                                        c0 : c0 + C_IN,
                                        h0 + di : h0 + di + RCHUNK,
                                        dj : dj + H,
                                    ],
                                    start=(k == 0),
                                    stop=last,
                                )
                                if last and half == 1:
                                    mm.then_inc(s_mm, 1)

            @block.vector
            def _(vector: bass.BassEngine):
                for p in range(2):
                    img = 2 * p
                    for ci in range(NCHUNK):
                        c = p * NCHUNK + ci
                        h0 = ci * RCHUNK
                        vector.wait_ge(s_mm, c + 1)
                        vector.tensor_copy(
                            out=obs[img][:, h0 : h0 + RCHUNK, :],
                            in_=banks[c % 4][0][:],
                        ).then_inc(s_cpv, 1)

    return nc


def _prep_inputs(x, K, mm_dt=MM_DT):
    np_dt = mybir.dt.np(mm_dt)
    x = np.ascontiguousarray(np.asarray(x, dtype=np.float32))
    K = np.ascontiguousarray(np.asarray(K, dtype=np.float32))
    xpad = np.pad(x, ((0, 0), (0, 0), (1, 1), (1, 1))).astype(np_dt)
    Wt = K.reshape(KS * KS * C_IN, C_OUT).reshape(C_IN, KS * KS, C_OUT)
    Wrep = np.ascontiguousarray(np.concatenate([Wt, Wt], axis=0)).astype(np_dt)
    shards = xpad.reshape(N_CORES, BPC, C_IN, HP, HP)
    return [{"x": np.ascontiguousarray(shards[i]), "w": Wrep} for i in range(N_CORES)]


def run(x, K, trace=False, mm_dt=MM_DT):
    nc = build_nc(mm_dt)
    in_maps = _prep_inputs(x, K, mm_dt)
    res = run_bass_kernel_spmd(nc, in_maps, list(range(N_CORES)), trace=trace)
    out = np.concatenate([res.results[i]["out"] for i in range(N_CORES)], axis=0)
    return out, res


def kernel(x, K):
    out, _ = run(x, K, trace=False)
    return out
